# revision 10
# baseline (speedup 1.0000x reference)
"""Trainium2 Bass kernel for nn_AddDropMRR (add-drop microring resonator).

Math: rotate the complex plane per wavelength by -arg(G) (magnitudes are
invariant), where G = t2*s1/den is the ring response. With u = P*x + s*a
and v = (r+P)*x + s*a:

  through = sqrt(g^2*v^2 + c2^2*x^2)        [all coefs per-wavelength]
  drop    = k2c * sqrt(u^2 + Q^2*x^2)

All per-wavelength coefficients depend only on `wavelengths` (8192 values)
and scalar params -> computed on HOST, DMA'd as small f32/f16 tables. The
device graph is pure streaming, software-pipelined in 5 skewed stages so no
engine queue blocks on a same-round cross-engine dependency:

  s0 DMA(qSP):  load x, a' chunk                      (~3.5us/chunk)
  s1 DVE:       u, v via TS-ptr (4x fp16) + TT (2x)   (~4.1us)
  s2 DVE:       u^2, v^2, x^2 in place                (~3.6us)
  s3 PE:        W2 = diag(g^2)@vv + diag(c2^2)@xx,
                D2 = diag(Q^2)@xx + I@uu  -> PSUM     (~5us, 16 matmuls)
  s4 ACT:       4x Sqrt([128,1024] PSUM half) -> fp16, output DMAs (qAct)

GPSIMD is deliberately idle: its SBUF port is shared with DVE's 2-port
perf modes, so gpsimd work serializes against the 4x TS ops (measured).
Tensors ride fp16 (2-byte dtype enables the DVE fast modes; better
mantissa than bf16). PSUM accumulates in f32 so no overflow rescaling is
needed. Sharding: wavelength dim split 8 ways across cores (data-parallel,
fully elementwise); host transposes so wavelength lies on SBUF partitions.
"""
import numpy as np

B = 2048           # batch
W = 8192           # wavelengths
NCORES = 8
WSH = W // NCORES  # 1024 wavelengths per core
P = 128            # SBUF partitions
NCHUNK = WSH // P  # 8 chunks per core
NCOEF = 5          # P, r+P, g/32, (c2/32)^2, Q^2
N_EFF = 2.4
CIRC = 2.0 * np.pi * 1e-05
MODE = "b"         # 'b': W2/D2 on PE;  'v': v on PE, adds on DVE


def _host_prep(wavelengths, coupling_1, coupling_2, phi_1, phi_2, phi_ring,
               alpha):
    """Scalars + per-wavelength coefficient vectors (f64 -> f32)."""
    c1 = float(np.asarray(coupling_1).reshape(-1)[0])
    c2 = float(np.asarray(coupling_2).reshape(-1)[0])
    p1 = float(np.asarray(phi_1).reshape(-1)[0])
    pr = float(np.asarray(phi_ring).reshape(-1)[0])
    al = float(np.asarray(alpha).reshape(-1)[0])
    k1c = float(np.clip(c1, 0.01, 0.99))
    k2c = float(np.clip(c2, 0.01, 0.99))
    t1 = float(np.sqrt(1.0 - k1c * k1c))
    t2 = float(np.sqrt(1.0 - k2c * k2c))
    s = float(np.sqrt(c2))       # unclamped, as in reference
    s1 = float(np.sqrt(c1))      # unclamped
    kappa = float(al * np.sqrt(1.0 - c1 * c1) * np.sqrt(1.0 - c2 * c2))

    # phi in f32 exactly as the reference computes it, then f64 trig
    wl = np.asarray(wavelengths, np.float32)
    phi32 = (np.float32(2.0 * np.pi * N_EFF) / wl) * np.float32(CIRC) \
        + np.float32(pr)
    phi = phi32.astype(np.float64)
    sin_p = np.sin(phi + p1)
    cos_p = np.cos(phi + p1)
    sin_f = np.sin(phi)
    cos_f = np.cos(phi)

    Pv = -k1c * al * sin_p
    Qv = k1c * al * cos_p
    den_re = 1.0 - kappa * cos_f
    den2 = den_re * den_re + (kappa * sin_f) ** 2
    rsq = 1.0 / np.sqrt(den2)
    g = (t2 * s1) * rsq
    r = (t1 / (t2 * s1)) * den_re
    c2v = (t2 * s1 * Qv - t1 * kappa * sin_f) * rsq

    coefs = np.stack([
        Pv,
        r + Pv,
        g / 32.0,
        (c2v / 32.0) ** 2,
        Qv ** 2,
    ]).astype(np.float32)                       # [NCOEF, W]
    vecs = dict(g2=(g * g), c22=(c2v * c2v), q2=(Qv * Qv), rp=(r + Pv))
    return coefs, dict(s=s, k2c=k2c, **{k: v.astype(np.float32)
                                        for k, v in vecs.items()})


def _build_graph(k2c, loop_n=1, nchunk=NCHUNK, bufs=8, mode=MODE,
                 split_dma=True):
    """SPMD per-core graph; see module docstring. loop_n>1 wraps the body
    in an on-device For_i loop for steady-state timing."""
    import concourse.tile as tile
    from concourse import bacc, mybir, bass

    f32 = mybir.dt.float32
    f16 = mybir.dt.float16
    AF = mybir.ActivationFunctionType
    ALU = mybir.AluOpType

    wsh = nchunk * P
    ndiag = (3 * nchunk + 1) if mode == "b" else (nchunk + 1)
    nc = bacc.Bacc("TRN2", target_bir_lowering=False, debug=False,
                   num_devices=NCORES)
    x_ext = nc.declare_dram_parameter("x_t", [wsh, B], f16, isOutput=False)
    a_ext = nc.declare_dram_parameter("a_t", [wsh, B], f16, isOutput=False)
    cf_ext = nc.declare_dram_parameter("cf_t", [P, NCOEF * nchunk], f32,
                                       isOutput=False)
    dg_ext = nc.declare_dram_parameter("dg_t", [P, ndiag * P], f16,
                                       isOutput=False)
    o1_ext = nc.declare_dram_parameter("o1_t", [wsh, B], f16, isOutput=True)
    o2_ext = nc.declare_dram_parameter("o2_t", [wsh, B], f16, isOutput=True)

    k2sq = float(k2c * k2c)
    out_eng = "scalar" if split_dma else "sync"
    HB = B // 2  # psum half width

    with tile.TileContext(nc) as tc:
        with tc.tile_pool(name="cst", bufs=1) as cst, \
             tc.tile_pool(name="mio", bufs=bufs) as mio, \
             tc.tile_pool(name="psum", bufs=2,
                          space=bass.MemorySpace.PSUM) as psum:

            def body(_iv=None):
                cf = cst.tile([P, NCOEF * nchunk], f32, tag="cf", name="cf")
                nc.sync.dma_start(cf[:], cf_ext[:])
                dg = cst.tile([P, ndiag * P], f16, tag="dg", name="dg")
                nc.sync.dma_start(dg[:], dg_ext[:])

                def C(k, c):
                    return cf[:, k * nchunk + c:k * nchunk + c + 1]

                def DG(k):  # k-th [P,P] diag block
                    return dg[:, k * P:(k + 1) * P]

                st = {}

                def s0(c):
                    rs = slice(c * P, (c + 1) * P)
                    xt = mio.tile([P, B], f16, tag="xt", name="xt")
                    nc.sync.dma_start(xt[:], x_ext[rs, :])
                    at = mio.tile([P, B], f16, tag="at", name="at")
                    nc.sync.dma_start(at[:], a_ext[rs, :])
                    st[c] = dict(xt=xt, at=at)

                def s1(c):
                    d = st[c]
                    xt, at = d["xt"], d["at"]
                    ut = mio.tile([P, B], f16, tag="ut", name="ut")
                    nc.vector.tensor_scalar(ut[:], xt[:], C(0, c), None,
                                            ALU.mult)
                    nc.vector.tensor_add(ut[:], ut[:], at[:])
                    d["ut"] = ut
                    if mode == "b":
                        vt = mio.tile([P, B], f16, tag="vt", name="vt")
                        nc.vector.tensor_scalar(vt[:], xt[:], C(1, c), None,
                                                ALU.mult)
                        nc.vector.tensor_add(vt[:], vt[:], at[:])
                        d["vt"] = vt
                        # squares in place (same queue, stays one stage)
                        nc.vector.tensor_mul(ut[:], ut[:], ut[:])   # uu
                        nc.vector.tensor_mul(vt[:], vt[:], vt[:])   # vv
                        nc.vector.tensor_mul(xt[:], xt[:], xt[:])   # xx
                    else:
                        vps = psum.tile([P, B], f32, tag="vps", name="vps")
                        d["vps"] = vps
                        for j in range(0, B, 512):
                            nc.tensor.matmul(vps[:, j:j + 512], DG(c),
                                             xt[:, j:j + 512],
                                             start=True, stop=False)
                        for j in range(0, B, 512):
                            nc.tensor.matmul(vps[:, j:j + 512], DG(nchunk),
                                             at[:, j:j + 512],
                                             start=False, stop=True)

                def s2(c):
                    d = st[c]
                    xt, ut = d["xt"], d["ut"]
                    if mode == "b":
                        return  # folded into s1 (same engine, in order)
                    nc.vector.tensor_mul(ut[:], ut[:], ut[:])   # uu
                    vt = mio.tile([P, B], f16, tag="vt", name="vt")
                    d["vt"] = vt
                    nc.scalar.activation(vt[:], d["vps"][:], AF.Square,
                                         scale=C(2, c))
                    nc.vector.tensor_mul(xt[:], xt[:], xt[:])   # xx

                def s3(c):
                    d = st[c]
                    xt, ut, vt = d["xt"], d["ut"], d["vt"]
                    if mode == "b":
                        # W2 halves: diag(g2) @ vv  (+)  diag(c22) @ xx
                        # D2 halves: diag(q2) @ xx  (+)  I @ uu
                        wps = [psum.tile([P, HB], f32, tag="wp", name="wp")
                               for _ in range(2)]
                        dps = [psum.tile([P, HB], f32, tag="dp", name="dp")
                               for _ in range(2)]
                        d["wps"], d["dps"] = wps, dps
                        mm = nc.tensor.matmul

                        def sweep(dst, dgb, src, start, stop):
                            for h in range(2):
                                for j in range(0, HB, 512):
                                    mm(dst[h][:, j:j + 512], dgb,
                                       src[:, h * HB + j:h * HB + j + 512],
                                       start=start, stop=stop)

                        sweep(wps, DG(c), vt, True, False)
                        sweep(wps, DG(nchunk + c), xt, False, True)
                        sweep(dps, DG(2 * nchunk + c), xt, True, False)
                        sweep(dps, DG(3 * nchunk), ut, False, True)
                    else:
                        w2 = mio.tile([P, B], f16, tag="w2", name="w2")
                        nc.vector.tensor_scalar(w2[:], xt[:], C(3, c), None,
                                                ALU.mult)
                        nc.vector.tensor_add(vt[:], vt[:], w2[:])
                        nc.vector.tensor_scalar(xt[:], xt[:], C(4, c), None,
                                                ALU.mult)
                        nc.vector.tensor_add(ut[:], ut[:], xt[:])

                def s4(c):
                    d = st.pop(c)
                    rs = slice(c * P, (c + 1) * P)
                    ut, vt = d["ut"], d["vt"]
                    if mode == "b":
                        for h in range(2):
                            hs = slice(h * HB, (h + 1) * HB)
                            nc.scalar.activation(vt[:, hs], d["wps"][h][:],
                                                 AF.Sqrt)
                            nc.scalar.activation(ut[:, hs], d["dps"][h][:],
                                                 AF.Sqrt, scale=k2sq)
                    else:
                        nc.scalar.activation(vt[:], vt[:], AF.Sqrt,
                                             scale=1024.0)
                        nc.scalar.activation(ut[:], ut[:], AF.Sqrt,
                                             scale=k2sq)
                    getattr(nc, out_eng).dma_start(o1_ext[rs, :], vt[:])
                    getattr(nc, out_eng).dma_start(o2_ext[rs, :], ut[:])

                stages = ([s0, s1, s3, s4] if mode == "b"
                          else [s0, s1, s2, s3, s4])
                nstg = len(stages)
                for t in range(nchunk + nstg - 1):
                    for s in range(nstg - 1, -1, -1):
                        c = t - s
                        if 0 <= c < nchunk:
                            stages[s](c)

            if loop_n > 1:
                with tc.For_i(0, loop_n, 1):
                    body()
            else:
                body()

    nc.compile()
    return nc


def _shard_inputs(input_signal, add_signal, coefs, s, vecs=None, mode=MODE):
    x = np.asarray(input_signal, dtype=np.float32).astype(np.float16)
    a = (np.asarray(add_signal, dtype=np.float32)
         * np.float32(s)).astype(np.float16)
    vecs = vecs or {}
    in_maps = []
    for i in range(NCORES):
        sl = slice(i * WSH, (i + 1) * WSH)
        # coef layout [P, NCOEF*NCHUNK]: column k*NCHUNK+c holds coef k of
        # chunk c; row p is wavelength c*P+p within the shard.
        cf = np.ascontiguousarray(
            coefs[:, sl].reshape(NCOEF, NCHUNK, P)
            .transpose(2, 0, 1).reshape(P, NCOEF * NCHUNK))
        m = {
            "x_t": np.ascontiguousarray(x[:, sl].T),
            "a_t": np.ascontiguousarray(a[:, sl].T),
            "cf_t": cf,
        }

        def diag_blocks(names):
            nd = len(names) * NCHUNK + 1
            dgm = np.zeros((P, nd * P), np.float16)
            for k, nm in enumerate(names):
                vsh = np.asarray(vecs[nm])[sl].reshape(NCHUNK, P)
                for c in range(NCHUNK):
                    blk = k * NCHUNK + c
                    dgm[:, blk * P:(blk + 1) * P] = np.diag(
                        vsh[c].astype(np.float16))
            dgm[:, (nd - 1) * P:] = np.eye(P, dtype=np.float16)
            return dgm

        if mode == "b":
            m["dg_t"] = diag_blocks(["g2", "c22", "q2"])
        else:
            m["dg_t"] = diag_blocks(["rp"])
        in_maps.append(m)
    return in_maps


def _gather_outputs(results):
    through = np.empty((B, W), np.float32)
    drop = np.empty((B, W), np.float32)
    for i in range(NCORES):
        sl = slice(i * WSH, (i + 1) * WSH)
        through[:, sl] = results[i]["o1_t"].T.astype(np.float32)
        drop[:, sl] = results[i]["o2_t"].T.astype(np.float32)
    return through, drop


def kernel(input_signal, add_signal, wavelengths, coupling_1, coupling_2,
           phi_1, phi_2, phi_ring, alpha):
    from concourse.bass_utils import run_bass_kernel_spmd

    coefs, sc = _host_prep(wavelengths, coupling_1, coupling_2, phi_1, phi_2,
                           phi_ring, alpha)
    nc = _build_graph(sc["k2c"])
    in_maps = _shard_inputs(input_signal, add_signal, coefs, sc["s"], vecs=sc)
    res = run_bass_kernel_spmd(nc, in_maps, core_ids=list(range(NCORES)))
    return _gather_outputs(res.results)


# revision 12
# speedup vs baseline: 1.0179x; 1.0179x over previous
"""Trainium2 Bass kernel for nn_AddDropMRR (add-drop microring resonator).

Math: rotate the complex plane per wavelength by -arg(G) (magnitudes are
invariant), where G = t2*s1/den is the ring response. With u = P*x + s*a
and v = (r+P)*x + s*a:

  through = sqrt(g^2*v^2 + c2^2*x^2)        [all coefs per-wavelength]
  drop    = k2c * sqrt(u^2 + Q^2*x^2)

All per-wavelength coefficients depend only on `wavelengths` (8192 values)
and scalar params -> computed on HOST, DMA'd as small f32/f16 tables. The
device graph is pure streaming, software-pipelined in 5 skewed stages so no
engine queue blocks on a same-round cross-engine dependency:

  s0 DMA(qSP):  load x, a' chunk                      (~3.5us/chunk)
  s1 DVE:       u, v via TS-ptr (4x fp16) + TT (2x)   (~4.1us)
  s2 DVE:       u^2, v^2, x^2 in place                (~3.6us)
  s3 PE:        W2 = diag(g^2)@vv + diag(c2^2)@xx,
                D2 = diag(Q^2)@xx + I@uu  -> PSUM     (~5us, 16 matmuls)
  s4 ACT:       4x Sqrt([128,1024] PSUM half) -> fp16, output DMAs (qAct)

GPSIMD is deliberately idle: its SBUF port is shared with DVE's 2-port
perf modes, so gpsimd work serializes against the 4x TS ops (measured).
Tensors ride fp16 (2-byte dtype enables the DVE fast modes; better
mantissa than bf16). PSUM accumulates in f32 so no overflow rescaling is
needed. Sharding: wavelength dim split 8 ways across cores (data-parallel,
fully elementwise); host transposes so wavelength lies on SBUF partitions.
"""
import numpy as np

B = 2048           # batch
W = 8192           # wavelengths
NCORES = 8
WSH = W // NCORES  # 1024 wavelengths per core
P = 128            # SBUF partitions
NCHUNK = WSH // P  # 8 chunks per core
NCOEF = 5          # P, r+P, g/32, (c2/32)^2, Q^2
N_EFF = 2.4
CIRC = 2.0 * np.pi * 1e-05
MODE = "b"         # 'b': W2/D2 on PE;  'v': v on PE, adds on DVE


def _host_prep(wavelengths, coupling_1, coupling_2, phi_1, phi_2, phi_ring,
               alpha):
    """Scalars + per-wavelength coefficient vectors (f64 -> f32)."""
    c1 = float(np.asarray(coupling_1).reshape(-1)[0])
    c2 = float(np.asarray(coupling_2).reshape(-1)[0])
    p1 = float(np.asarray(phi_1).reshape(-1)[0])
    pr = float(np.asarray(phi_ring).reshape(-1)[0])
    al = float(np.asarray(alpha).reshape(-1)[0])
    k1c = float(np.clip(c1, 0.01, 0.99))
    k2c = float(np.clip(c2, 0.01, 0.99))
    t1 = float(np.sqrt(1.0 - k1c * k1c))
    t2 = float(np.sqrt(1.0 - k2c * k2c))
    s = float(np.sqrt(c2))       # unclamped, as in reference
    s1 = float(np.sqrt(c1))      # unclamped
    kappa = float(al * np.sqrt(1.0 - c1 * c1) * np.sqrt(1.0 - c2 * c2))

    # phi in f32 exactly as the reference computes it, then f64 trig
    wl = np.asarray(wavelengths, np.float32)
    phi32 = (np.float32(2.0 * np.pi * N_EFF) / wl) * np.float32(CIRC) \
        + np.float32(pr)
    phi = phi32.astype(np.float64)
    sin_p = np.sin(phi + p1)
    cos_p = np.cos(phi + p1)
    sin_f = np.sin(phi)
    cos_f = np.cos(phi)

    Pv = -k1c * al * sin_p
    Qv = k1c * al * cos_p
    den_re = 1.0 - kappa * cos_f
    den2 = den_re * den_re + (kappa * sin_f) ** 2
    rsq = 1.0 / np.sqrt(den2)
    g = (t2 * s1) * rsq
    r = (t1 / (t2 * s1)) * den_re
    c2v = (t2 * s1 * Qv - t1 * kappa * sin_f) * rsq

    coefs = np.stack([
        Pv,
        r + Pv,
        g / 32.0,
        (c2v / 32.0) ** 2,
        Qv ** 2,
    ]).astype(np.float32)                       # [NCOEF, W]
    vecs = dict(g2=(g * g), c22=(c2v * c2v), q2=(Qv * Qv), rp=(r + Pv))
    return coefs, dict(s=s, k2c=k2c, **{k: v.astype(np.float32)
                                        for k, v in vecs.items()})


def _build_graph(k2c, loop_n=1, nchunk=NCHUNK, bufs=8, mode=MODE,
                 split_dma=True, passes=1, taper=True):
    """SPMD per-core graph; see module docstring. loop_n>1 wraps the body
    in an on-device For_i loop for steady-state timing."""
    import concourse.tile as tile
    from concourse import bacc, mybir, bass

    f32 = mybir.dt.float32
    f16 = mybir.dt.float16
    AF = mybir.ActivationFunctionType
    ALU = mybir.AluOpType

    wsh = nchunk * P
    ndiag = (3 * nchunk + 1) if mode == "b" else (nchunk + 1)
    nc = bacc.Bacc("TRN2", target_bir_lowering=False, debug=False,
                   num_devices=NCORES)
    x_ext = nc.declare_dram_parameter("x_t", [wsh, B], f16, isOutput=False)
    a_ext = nc.declare_dram_parameter("a_t", [wsh, B], f16, isOutput=False)
    cf_ext = nc.declare_dram_parameter("cf_t", [P, NCOEF * nchunk], f32,
                                       isOutput=False)
    dg_ext = nc.declare_dram_parameter("dg_t", [P, ndiag * P], f16,
                                       isOutput=False)
    o1_ext = nc.declare_dram_parameter("o1_t", [wsh, B], f16, isOutput=True)
    o2_ext = nc.declare_dram_parameter("o2_t", [wsh, B], f16, isOutput=True)

    k2sq = float(k2c * k2c)
    out_eng = "scalar" if split_dma else "sync"
    HB = B // 2  # psum half width

    with tile.TileContext(nc) as tc:
        with tc.tile_pool(name="cst", bufs=1) as cst, \
             tc.tile_pool(name="mio", bufs=bufs) as mio, \
             tc.tile_pool(name="psum", bufs=2,
                          space=bass.MemorySpace.PSUM) as psum:

            def body(_iv=None):
                cf = cst.tile([P, NCOEF * nchunk], f32, tag="cf", name="cf")
                nc.sync.dma_start(cf[:], cf_ext[:])
                dg = cst.tile([P, ndiag * P], f16, tag="dg", name="dg")
                nc.sync.dma_start(dg[:], dg_ext[:])

                def C(k, c):
                    return cf[:, k * nchunk + c:k * nchunk + c + 1]

                def DG(k):  # k-th [P,P] diag block
                    return dg[:, k * P:(k + 1) * P]

                st = {}

                def s0(vc):
                    v_id, c, lo, hi = vc
                    rs = slice(c * P, (c + 1) * P)
                    cs = slice(lo, hi)
                    xt = mio.tile([P, B], f16, tag="xt", name="xt")
                    nc.sync.dma_start(xt[:, cs], x_ext[rs, cs])
                    at = mio.tile([P, B], f16, tag="at", name="at")
                    nc.sync.dma_start(at[:, cs], a_ext[rs, cs])
                    st[v_id] = dict(xt=xt, at=at)

                def s1(vc):
                    v_id, c, lo, hi = vc
                    cs = slice(lo, hi)
                    d = st[v_id]
                    xt, at = d["xt"], d["at"]
                    ut = mio.tile([P, B], f16, tag="ut", name="ut")
                    nc.vector.tensor_scalar(ut[:, cs], xt[:, cs], C(0, c),
                                            None, ALU.mult)
                    nc.vector.tensor_add(ut[:, cs], ut[:, cs], at[:, cs])
                    d["ut"] = ut
                    if mode == "b":
                        vt = mio.tile([P, B], f16, tag="vt", name="vt")
                        nc.vector.tensor_scalar(vt[:, cs], xt[:, cs], C(1, c),
                                                None, ALU.mult)
                        nc.vector.tensor_add(vt[:, cs], vt[:, cs], at[:, cs])
                        d["vt"] = vt
                        # squares in place (same queue, stays one stage)
                        nc.vector.tensor_mul(ut[:, cs], ut[:, cs], ut[:, cs])
                        nc.vector.tensor_mul(vt[:, cs], vt[:, cs], vt[:, cs])
                        nc.vector.tensor_mul(xt[:, cs], xt[:, cs], xt[:, cs])
                    else:
                        vps = psum.tile([P, B], f32, tag="vps", name="vps")
                        d["vps"] = vps
                        for j in range(0, B, 512):
                            nc.tensor.matmul(vps[:, j:j + 512], DG(c),
                                             xt[:, j:j + 512],
                                             start=True, stop=False)
                        for j in range(0, B, 512):
                            nc.tensor.matmul(vps[:, j:j + 512], DG(nchunk),
                                             at[:, j:j + 512],
                                             start=False, stop=True)

                def s2(vc):
                    v_id, c = vc
                    d = st[v_id]
                    xt, ut = d["xt"], d["ut"]
                    if mode == "b":
                        return  # folded into s1 (same engine, in order)
                    nc.vector.tensor_mul(ut[:], ut[:], ut[:])   # uu
                    vt = mio.tile([P, B], f16, tag="vt", name="vt")
                    d["vt"] = vt
                    nc.scalar.activation(vt[:], d["vps"][:], AF.Square,
                                         scale=C(2, c))
                    nc.vector.tensor_mul(xt[:], xt[:], xt[:])   # xx

                def s3(vc):
                    v_id, c, lo, hi = vc
                    d = st[v_id]
                    xt, ut, vt = d["xt"], d["ut"], d["vt"]
                    if mode == "b":
                        # W2 groups: diag(g2) @ vv  (+)  diag(c22) @ xx
                        # D2 groups: diag(q2) @ xx  (+)  I @ uu
                        groups = list(range(lo, hi, HB))
                        wps = [psum.tile([P, HB], f32, tag="wp", name="wp")
                               for _ in groups]
                        dps = [psum.tile([P, HB], f32, tag="dp", name="dp")
                               for _ in groups]
                        d["wps"], d["dps"], d["groups"] = wps, dps, groups
                        mm = nc.tensor.matmul

                        def sweep(dst, dgb, src, start, stop):
                            for h, goff in enumerate(groups):
                                for j in range(0, HB, 512):
                                    mm(dst[h][:, j:j + 512], dgb,
                                       src[:, goff + j:goff + j + 512],
                                       start=start, stop=stop)

                        sweep(wps, DG(c), vt, True, False)
                        sweep(wps, DG(nchunk + c), xt, False, True)
                        sweep(dps, DG(2 * nchunk + c), xt, True, False)
                        sweep(dps, DG(3 * nchunk), ut, False, True)
                    else:
                        w2 = mio.tile([P, B], f16, tag="w2", name="w2")
                        nc.vector.tensor_scalar(w2[:], xt[:], C(3, c), None,
                                                ALU.mult)
                        nc.vector.tensor_add(vt[:], vt[:], w2[:])
                        nc.vector.tensor_scalar(xt[:], xt[:], C(4, c), None,
                                                ALU.mult)
                        nc.vector.tensor_add(ut[:], ut[:], xt[:])

                def s4(vc):
                    v_id, c, lo, hi = vc
                    d = st.pop(v_id)
                    rs = slice(c * P, (c + 1) * P)
                    cs = slice(lo, hi)
                    ut, vt = d["ut"], d["vt"]
                    if mode == "b":
                        for h, goff in enumerate(d["groups"]):
                            hs = slice(goff, goff + HB)
                            nc.scalar.activation(vt[:, hs], d["wps"][h][:],
                                                 AF.Sqrt)
                            nc.scalar.activation(ut[:, hs], d["dps"][h][:],
                                                 AF.Sqrt, scale=k2sq)
                    else:
                        nc.scalar.activation(vt[:], vt[:], AF.Sqrt,
                                             scale=1024.0)
                        nc.scalar.activation(ut[:], ut[:], AF.Sqrt,
                                             scale=k2sq)
                    getattr(nc, out_eng).dma_start(o1_ext[rs, cs], vt[:, cs])
                    getattr(nc, out_eng).dma_start(o2_ext[rs, cs], ut[:, cs])

                stages = ([s0, s1, s3, s4] if mode == "b"
                          else [s0, s1, s2, s3, s4])
                nstg = len(stages)
                # virtual chunk list: first/last row-chunks split into
                # column halves so the pipeline fills fast and drains with a
                # short tail; middle chunks full-width (lowest op overhead).
                # passes>1 repeats chunks to amortize fill/drain in probes.
                spans = []
                for c in range(nchunk):
                    if taper and c == 0:
                        spans += [(c, 0, HB), (c, HB, B)]
                    elif taper and c == nchunk - 1:
                        spans += [(c, 0, HB), (c, HB, B)]
                    else:
                        spans.append((c, 0, B))
                vchunks = [(p * len(spans) + i, c, lo, hi)
                           for p in range(passes)
                           for i, (c, lo, hi) in enumerate(spans)]
                nv = len(vchunks)
                for t in range(nv + nstg - 1):
                    for s in range(nstg - 1, -1, -1):
                        i = t - s
                        if 0 <= i < nv:
                            stages[s](vchunks[i])

            if loop_n > 1:
                with tc.For_i(0, loop_n, 1):
                    body()
            else:
                body()

    nc.compile()
    return nc


def _shard_inputs(input_signal, add_signal, coefs, s, vecs=None, mode=MODE):
    x = np.asarray(input_signal, dtype=np.float32).astype(np.float16)
    a = (np.asarray(add_signal, dtype=np.float32)
         * np.float32(s)).astype(np.float16)
    vecs = vecs or {}
    in_maps = []
    for i in range(NCORES):
        sl = slice(i * WSH, (i + 1) * WSH)
        # coef layout [P, NCOEF*NCHUNK]: column k*NCHUNK+c holds coef k of
        # chunk c; row p is wavelength c*P+p within the shard.
        cf = np.ascontiguousarray(
            coefs[:, sl].reshape(NCOEF, NCHUNK, P)
            .transpose(2, 0, 1).reshape(P, NCOEF * NCHUNK))
        m = {
            "x_t": np.ascontiguousarray(x[:, sl].T),
            "a_t": np.ascontiguousarray(a[:, sl].T),
            "cf_t": cf,
        }

        def diag_blocks(names):
            nd = len(names) * NCHUNK + 1
            dgm = np.zeros((P, nd * P), np.float16)
            for k, nm in enumerate(names):
                vsh = np.asarray(vecs[nm])[sl].reshape(NCHUNK, P)
                for c in range(NCHUNK):
                    blk = k * NCHUNK + c
                    dgm[:, blk * P:(blk + 1) * P] = np.diag(
                        vsh[c].astype(np.float16))
            dgm[:, (nd - 1) * P:] = np.eye(P, dtype=np.float16)
            return dgm

        if mode == "b":
            m["dg_t"] = diag_blocks(["g2", "c22", "q2"])
        else:
            m["dg_t"] = diag_blocks(["rp"])
        in_maps.append(m)
    return in_maps


def _gather_outputs(results):
    through = np.empty((B, W), np.float32)
    drop = np.empty((B, W), np.float32)
    for i in range(NCORES):
        sl = slice(i * WSH, (i + 1) * WSH)
        through[:, sl] = results[i]["o1_t"].T.astype(np.float32)
        drop[:, sl] = results[i]["o2_t"].T.astype(np.float32)
    return through, drop


def kernel(input_signal, add_signal, wavelengths, coupling_1, coupling_2,
           phi_1, phi_2, phi_ring, alpha):
    from concourse.bass_utils import run_bass_kernel_spmd

    coefs, sc = _host_prep(wavelengths, coupling_1, coupling_2, phi_1, phi_2,
                           phi_ring, alpha)
    nc = _build_graph(sc["k2c"])
    in_maps = _shard_inputs(input_signal, add_signal, coefs, sc["s"], vecs=sc)
    res = run_bass_kernel_spmd(nc, in_maps, core_ids=list(range(NCORES)))
    return _gather_outputs(res.results)


# revision 15
# speedup vs baseline: 1.0591x; 1.0404x over previous
"""Trainium2 Bass kernel for nn_AddDropMRR (add-drop microring resonator).

Math: rotate the complex plane per wavelength by -arg(G) (magnitudes are
invariant), where G = t2*s1/den is the ring response. With u = P*x + s*a
and v = (r+P)*x + s*a:

  through = sqrt(g^2*v^2 + c2^2*x^2)        [all coefs per-wavelength]
  drop    = k2c * sqrt(u^2 + Q^2*x^2)

All per-wavelength coefficients depend only on `wavelengths` (8192 values)
and scalar params -> computed on HOST, DMA'd as small f32/f16 tables. The
device graph is pure streaming, software-pipelined in 5 skewed stages so no
engine queue blocks on a same-round cross-engine dependency:

  s0 DMA(qSP):  load x, a' chunk                      (~3.5us/chunk)
  s1 DVE:       u, v via TS-ptr (4x fp16) + TT (2x)   (~4.1us)
  s2 DVE:       u^2, v^2, x^2 in place                (~3.6us)
  s3 PE:        W2 = diag(g^2)@vv + diag(c2^2)@xx,
                D2 = diag(Q^2)@xx + I@uu  -> PSUM     (~5us, 16 matmuls)
  s4 ACT:       4x Sqrt([128,1024] PSUM half) -> fp16, output DMAs (qAct)

GPSIMD is deliberately idle: its SBUF port is shared with DVE's 2-port
perf modes, so gpsimd work serializes against the 4x TS ops (measured).
Tensors ride fp16 (2-byte dtype enables the DVE fast modes; better
mantissa than bf16). PSUM accumulates in f32 so no overflow rescaling is
needed. Sharding: wavelength dim split 8 ways across cores (data-parallel,
fully elementwise); host transposes so wavelength lies on SBUF partitions.
"""
import numpy as np

B = 2048           # batch
W = 8192           # wavelengths
NCORES = 8
WSH = W // NCORES  # 1024 wavelengths per core
P = 128            # SBUF partitions
NCHUNK = WSH // P  # 8 chunks per core
NCOEF = 5          # P, r+P, g/32, (c2/32)^2, Q^2
N_EFF = 2.4
CIRC = 2.0 * np.pi * 1e-05
MODE = "b"         # 'b': W2/D2 on PE;  'v': v on PE, adds on DVE


def _host_prep(wavelengths, coupling_1, coupling_2, phi_1, phi_2, phi_ring,
               alpha):
    """Scalars + per-wavelength coefficient vectors (f64 -> f32)."""
    c1 = float(np.asarray(coupling_1).reshape(-1)[0])
    c2 = float(np.asarray(coupling_2).reshape(-1)[0])
    p1 = float(np.asarray(phi_1).reshape(-1)[0])
    pr = float(np.asarray(phi_ring).reshape(-1)[0])
    al = float(np.asarray(alpha).reshape(-1)[0])
    k1c = float(np.clip(c1, 0.01, 0.99))
    k2c = float(np.clip(c2, 0.01, 0.99))
    t1 = float(np.sqrt(1.0 - k1c * k1c))
    t2 = float(np.sqrt(1.0 - k2c * k2c))
    s = float(np.sqrt(c2))       # unclamped, as in reference
    s1 = float(np.sqrt(c1))      # unclamped
    kappa = float(al * np.sqrt(1.0 - c1 * c1) * np.sqrt(1.0 - c2 * c2))

    # phi in f32 exactly as the reference computes it, then f64 trig
    wl = np.asarray(wavelengths, np.float32)
    phi32 = (np.float32(2.0 * np.pi * N_EFF) / wl) * np.float32(CIRC) \
        + np.float32(pr)
    phi = phi32.astype(np.float64)
    sin_p = np.sin(phi + p1)
    cos_p = np.cos(phi + p1)
    sin_f = np.sin(phi)
    cos_f = np.cos(phi)

    Pv = -k1c * al * sin_p
    Qv = k1c * al * cos_p
    den_re = 1.0 - kappa * cos_f
    den2 = den_re * den_re + (kappa * sin_f) ** 2
    rsq = 1.0 / np.sqrt(den2)
    g = (t2 * s1) * rsq
    r = (t1 / (t2 * s1)) * den_re
    c2v = (t2 * s1 * Qv - t1 * kappa * sin_f) * rsq

    coefs = np.stack([
        Pv,
        r + Pv,
        g / 32.0,
        (c2v / 32.0) ** 2,
        Qv ** 2,
    ]).astype(np.float32)                       # [NCOEF, W]
    vecs = dict(g2=(g * g), c22=(c2v * c2v), q2=(Qv * Qv), rp=(r + Pv))
    return coefs, dict(s=s, k2c=k2c, **{k: v.astype(np.float32)
                                        for k, v in vecs.items()})


def _build_graph(k2c, loop_n=1, nchunk=NCHUNK, bufs=10, mode=MODE,
                 split_dma=True, passes=1, taper=1, swdge_o2=True):
    """SPMD per-core graph; see module docstring. loop_n>1 wraps the body
    in an on-device For_i loop for steady-state timing."""
    import concourse.tile as tile
    from concourse import bacc, mybir, bass

    f32 = mybir.dt.float32
    f16 = mybir.dt.float16
    AF = mybir.ActivationFunctionType
    ALU = mybir.AluOpType

    wsh = nchunk * P
    ndiag = (3 * nchunk + 1) if mode == "b" else (nchunk + 1)
    nc = bacc.Bacc("TRN2", target_bir_lowering=False, debug=False,
                   num_devices=NCORES)
    x_ext = nc.declare_dram_parameter("x_t", [wsh, B], f16, isOutput=False)
    a_ext = nc.declare_dram_parameter("a_t", [wsh, B], f16, isOutput=False)
    cf_ext = nc.declare_dram_parameter("cf_t", [P, NCOEF * nchunk], f32,
                                       isOutput=False)
    dg_ext = nc.declare_dram_parameter("dg_t", [P, ndiag * P], f16,
                                       isOutput=False)
    o1_ext = nc.declare_dram_parameter("o1_t", [wsh, B], f16, isOutput=True)
    o2_ext = nc.declare_dram_parameter("o2_t", [wsh, B], f16, isOutput=True)

    k2sq = float(k2c * k2c)
    out_eng = "scalar" if split_dma else "sync"
    HB = B // 2  # psum half width

    with tile.TileContext(nc) as tc:
        with tc.tile_pool(name="cst", bufs=1) as cst, \
             tc.tile_pool(name="mio", bufs=bufs) as mio, \
             tc.tile_pool(name="psum", bufs=2,
                          space=bass.MemorySpace.PSUM) as psum:

            def body(_iv=None):
                cf = cst.tile([P, NCOEF * nchunk], f32, tag="cf", name="cf")
                nc.sync.dma_start(cf[:], cf_ext[:])
                dg = cst.tile([P, ndiag * P], f16, tag="dg", name="dg")
                nc.sync.dma_start(dg[:], dg_ext[:])

                def C(k, c):
                    return cf[:, k * nchunk + c:k * nchunk + c + 1]

                def DG(k):  # k-th [P,P] diag block
                    return dg[:, k * P:(k + 1) * P]

                st = {}

                def s0(vc):
                    v_id, c, lo, hi = vc
                    rs = slice(c * P, (c + 1) * P)
                    cs = slice(lo, hi)
                    xt = mio.tile([P, B], f16, tag="xt", name="xt")
                    nc.sync.dma_start(xt[:, cs], x_ext[rs, cs])
                    at = mio.tile([P, B], f16, tag="at", name="at")
                    nc.sync.dma_start(at[:, cs], a_ext[rs, cs])
                    st[v_id] = dict(xt=xt, at=at)

                def s1(vc):
                    v_id, c, lo, hi = vc
                    cs = slice(lo, hi)
                    d = st[v_id]
                    xt, at = d["xt"], d["at"]
                    ut = mio.tile([P, B], f16, tag="ut", name="ut")
                    nc.vector.tensor_scalar(ut[:, cs], xt[:, cs], C(0, c),
                                            None, ALU.mult)
                    nc.vector.tensor_add(ut[:, cs], ut[:, cs], at[:, cs])
                    d["ut"] = ut
                    if mode == "b":
                        vt = mio.tile([P, B], f16, tag="vt", name="vt")
                        nc.vector.tensor_scalar(vt[:, cs], xt[:, cs], C(1, c),
                                                None, ALU.mult)
                        nc.vector.tensor_add(vt[:, cs], vt[:, cs], at[:, cs])
                        d["vt"] = vt
                        # squares in place (same queue, stays one stage)
                        nc.vector.tensor_mul(ut[:, cs], ut[:, cs], ut[:, cs])
                        nc.vector.tensor_mul(vt[:, cs], vt[:, cs], vt[:, cs])
                        nc.vector.tensor_mul(xt[:, cs], xt[:, cs], xt[:, cs])
                    else:
                        vps = psum.tile([P, B], f32, tag="vps", name="vps")
                        d["vps"] = vps
                        for j in range(0, B, 512):
                            nc.tensor.matmul(vps[:, j:j + 512], DG(c),
                                             xt[:, j:j + 512],
                                             start=True, stop=False)
                        for j in range(0, B, 512):
                            nc.tensor.matmul(vps[:, j:j + 512], DG(nchunk),
                                             at[:, j:j + 512],
                                             start=False, stop=True)

                def s2(vc):
                    v_id, c = vc
                    d = st[v_id]
                    xt, ut = d["xt"], d["ut"]
                    if mode == "b":
                        return  # folded into s1 (same engine, in order)
                    nc.vector.tensor_mul(ut[:], ut[:], ut[:])   # uu
                    vt = mio.tile([P, B], f16, tag="vt", name="vt")
                    d["vt"] = vt
                    nc.scalar.activation(vt[:], d["vps"][:], AF.Square,
                                         scale=C(2, c))
                    nc.vector.tensor_mul(xt[:], xt[:], xt[:])   # xx

                def s3(vc):
                    v_id, c, lo, hi = vc
                    d = st[v_id]
                    xt, ut, vt = d["xt"], d["ut"], d["vt"]
                    if mode == "b":
                        # W2 groups: diag(g2) @ vv  (+)  diag(c22) @ xx
                        # D2 groups: diag(q2) @ xx  (+)  I @ uu
                        groups = [(goff, min(HB, hi - goff))
                                  for goff in range(lo, hi, HB)]
                        wps = [psum.tile([P, HB], f32, tag="wp", name="wp")
                               for _ in groups]
                        dps = [psum.tile([P, HB], f32, tag="dp", name="dp")
                               for _ in groups]
                        d["wps"], d["dps"], d["groups"] = wps, dps, groups
                        mm = nc.tensor.matmul

                        def sweep(dst, dgb, src, start, stop):
                            for h, (goff, gw) in enumerate(groups):
                                for j in range(0, gw, 512):
                                    w = min(512, gw - j)
                                    mm(dst[h][:, j:j + w], dgb,
                                       src[:, goff + j:goff + j + w],
                                       start=start, stop=stop)

                        sweep(wps, DG(c), vt, True, False)
                        sweep(wps, DG(nchunk + c), xt, False, True)
                        sweep(dps, DG(2 * nchunk + c), xt, True, False)
                        sweep(dps, DG(3 * nchunk), ut, False, True)
                    else:
                        w2 = mio.tile([P, B], f16, tag="w2", name="w2")
                        nc.vector.tensor_scalar(w2[:], xt[:], C(3, c), None,
                                                ALU.mult)
                        nc.vector.tensor_add(vt[:], vt[:], w2[:])
                        nc.vector.tensor_scalar(xt[:], xt[:], C(4, c), None,
                                                ALU.mult)
                        nc.vector.tensor_add(ut[:], ut[:], xt[:])

                def s4(vc):
                    v_id, c, lo, hi = vc
                    d = st.pop(v_id)
                    rs = slice(c * P, (c + 1) * P)
                    cs = slice(lo, hi)
                    ut, vt = d["ut"], d["vt"]
                    if mode == "b":
                        for h, (goff, gw) in enumerate(d["groups"]):
                            hs = slice(goff, goff + gw)
                            nc.scalar.activation(vt[:, hs],
                                                 d["wps"][h][:, 0:gw],
                                                 AF.Sqrt)
                            nc.scalar.activation(ut[:, hs],
                                                 d["dps"][h][:, 0:gw],
                                                 AF.Sqrt, scale=k2sq)
                    else:
                        nc.scalar.activation(vt[:], vt[:], AF.Sqrt,
                                             scale=1024.0)
                        nc.scalar.activation(ut[:], ut[:], AF.Sqrt,
                                             scale=k2sq)
                    getattr(nc, out_eng).dma_start(o1_ext[rs, cs], vt[:, cs])
                    o2q = nc.gpsimd if swdge_o2 else getattr(nc, out_eng)
                    o2q.dma_start(o2_ext[rs, cs], ut[:, cs])

                stages = ([s0, s1, s3, s4] if mode == "b"
                          else [s0, s1, s2, s3, s4])
                nstg = len(stages)
                # virtual chunk list: first/last row-chunks split into
                # column halves so the pipeline fills fast and drains with a
                # short tail; middle chunks full-width (lowest op overhead).
                # passes>1 repeats chunks to amortize fill/drain in probes.
                spans = []
                for c in range(nchunk):
                    if taper >= 2 and c == 0:
                        spans += [(c, 0, 512), (c, 512, 1024), (c, 1024, B)]
                    elif taper >= 2 and c == nchunk - 1:
                        spans += [(c, 0, 1024), (c, 1024, 1536),
                                  (c, 1536, B)]
                    elif taper == 1 and c in (0, nchunk - 1):
                        spans += [(c, 0, HB), (c, HB, B)]
                    else:
                        spans.append((c, 0, B))
                vchunks = [(p * len(spans) + i, c, lo, hi)
                           for p in range(passes)
                           for i, (c, lo, hi) in enumerate(spans)]
                nv = len(vchunks)
                for t in range(nv + nstg - 1):
                    for s in range(nstg - 1, -1, -1):
                        i = t - s
                        if 0 <= i < nv:
                            stages[s](vchunks[i])

            if loop_n > 1:
                with tc.For_i(0, loop_n, 1):
                    body()
            else:
                body()

    nc.compile()
    return nc


def _shard_inputs(input_signal, add_signal, coefs, s, vecs=None, mode=MODE):
    x = np.asarray(input_signal, dtype=np.float32).astype(np.float16)
    a = (np.asarray(add_signal, dtype=np.float32)
         * np.float32(s)).astype(np.float16)
    vecs = vecs or {}
    in_maps = []
    for i in range(NCORES):
        sl = slice(i * WSH, (i + 1) * WSH)
        # coef layout [P, NCOEF*NCHUNK]: column k*NCHUNK+c holds coef k of
        # chunk c; row p is wavelength c*P+p within the shard.
        cf = np.ascontiguousarray(
            coefs[:, sl].reshape(NCOEF, NCHUNK, P)
            .transpose(2, 0, 1).reshape(P, NCOEF * NCHUNK))
        m = {
            "x_t": np.ascontiguousarray(x[:, sl].T),
            "a_t": np.ascontiguousarray(a[:, sl].T),
            "cf_t": cf,
        }

        def diag_blocks(names):
            nd = len(names) * NCHUNK + 1
            dgm = np.zeros((P, nd * P), np.float16)
            for k, nm in enumerate(names):
                vsh = np.asarray(vecs[nm])[sl].reshape(NCHUNK, P)
                for c in range(NCHUNK):
                    blk = k * NCHUNK + c
                    dgm[:, blk * P:(blk + 1) * P] = np.diag(
                        vsh[c].astype(np.float16))
            dgm[:, (nd - 1) * P:] = np.eye(P, dtype=np.float16)
            return dgm

        if mode == "b":
            m["dg_t"] = diag_blocks(["g2", "c22", "q2"])
        else:
            m["dg_t"] = diag_blocks(["rp"])
        in_maps.append(m)
    return in_maps


def _gather_outputs(results):
    through = np.empty((B, W), np.float32)
    drop = np.empty((B, W), np.float32)
    for i in range(NCORES):
        sl = slice(i * WSH, (i + 1) * WSH)
        through[:, sl] = results[i]["o1_t"].T.astype(np.float32)
        drop[:, sl] = results[i]["o2_t"].T.astype(np.float32)
    return through, drop


def kernel(input_signal, add_signal, wavelengths, coupling_1, coupling_2,
           phi_1, phi_2, phi_ring, alpha):
    from concourse.bass_utils import run_bass_kernel_spmd

    coefs, sc = _host_prep(wavelengths, coupling_1, coupling_2, phi_1, phi_2,
                           phi_ring, alpha)
    nc = _build_graph(sc["k2c"])
    in_maps = _shard_inputs(input_signal, add_signal, coefs, sc["s"], vecs=sc)
    res = run_bass_kernel_spmd(nc, in_maps, core_ids=list(range(NCORES)))
    return _gather_outputs(res.results)
